# revision 1
# baseline (speedup 1.0000x reference)
"""Trainium2 Bass kernel for nn_ExtendedSympNet (Suzuki-4 composition of
extended symplectic verlet steps driven by a 6-layer MLP Hamiltonian).

Strategy: pure data parallel over 8 NeuronCores (4096 samples each).
Activations are kept feature-major [512 feat (partitions), 512 batch (free)];
each of the 10 gradient evaluations is a fused forward+backward pass of the
MLP done fully on-chip. Matmuls run in float32r (full PE rate at N=512),
the integrator state stays in float32.

Self-contained: hardcodes all shapes from the problem spec.
"""
import os

# the Bass kernel executes through the axon PJRT backend; make sure a
# CPU-pinned JAX_PLATFORMS doesn't hide the NeuronCores
if os.environ.get("JAX_PLATFORMS", "").strip() == "cpu":
    os.environ["JAX_PLATFORMS"] = "axon,cpu"

import numpy as np
from contextlib import ExitStack

import concourse.bacc as bacc
import concourse.bass as bass
import concourse.mybir as mybir
import concourse.tile as tile
from concourse.bass_utils import run_bass_kernel_spmd

F32 = mybir.dt.float32
F32R = mybir.dt.float32r
AF = mybir.ActivationFunctionType
ALU = mybir.AluOpType

B, LAT, HID = 32768, 64, 512
N_CORES = 8
BC = B // N_CORES          # samples per core = 4096
BT = 512                   # batch tile (matmul moving dim / PSUM bank)
NBT = BC // BT             # 8 batch tiles per core
DT = 0.1
NSTEP = 5                  # Suzuki composition sub-steps
NL = 5                     # tanh layers in H-net


def _pack_k(w: np.ndarray) -> np.ndarray:
    """[512, C] -> [128, 4*C]: 128-row k-tile blocks side by side."""
    assert w.shape[0] == 4 * 128
    return np.concatenate([w[k * 128:(k + 1) * 128, :] for k in range(4)], axis=1)


def build_program(n_bt: int = NBT, n_step: int = NSTEP):
    nc = bacc.Bacc("TRN2", target_bir_lowering=False, debug=False)

    # ---- DRAM io ----
    d = {}
    # active rows of z (f32 state); aux rows live only in the f32r mirror
    d["zf"] = nc.dram_tensor("zf", [4, BC], F32, kind="ExternalInput").ap()
    # z mirror duplicated across both partition halves (for layer-1 row packing)
    d["zr"] = nc.dram_tensor("zr", [128, BC], F32R, kind="ExternalInput").ap()
    d["w1"] = nc.dram_tensor("w1", [128, HID], F32R, kind="ExternalInput").ap()
    # forward weights W2..W5 packed: [128, 4 layers * 2048]
    d["wf"] = nc.dram_tensor("wf", [128, 4 * 4 * HID], F32R, kind="ExternalInput").ap()
    # backward weights W2^T..W5^T packed the same way
    d["wb"] = nc.dram_tensor("wb", [128, 4 * 4 * HID], F32R, kind="ExternalInput").ap()
    # W1[:4,:].T, columns permuted [2,3,0,1] and pre-scaled by the per-half-step
    # gradient coefficients; packed per half-step: [128, 16 * 2*NSTEP]
    d["wga"] = nc.dram_tensor("wga", [128, 16 * 2 * NSTEP], F32R, kind="ExternalInput").ap()
    d["w6"] = nc.dram_tensor("w6", [128, 4], F32, kind="ExternalInput").ap()
    d["bia"] = nc.dram_tensor("bia", [128, 4 * NL], F32, kind="ExternalInput").ap()
    # per half-step active-update matrices A_hs [4,4] packed: [4, 4*2*NSTEP]
    d["smp"] = nc.dram_tensor("smp", [4, 4 * 2 * NSTEP], F32R, kind="ExternalInput").ap()
    zoa = nc.dram_tensor("zoa", [4, BC], F32, kind="ExternalOutput").ap()
    zox = nc.dram_tensor("zox", [LAT - 4, BC], F32R, kind="ExternalOutput").ap()

    with tile.TileContext(nc) as tc, ExitStack() as ctx:
        wpool = ctx.enter_context(tc.tile_pool(name="wpool", bufs=1))
        hpool = ctx.enter_context(tc.tile_pool(name="hpool", bufs=10))
        tpool = ctx.enter_context(tc.tile_pool(name="tpool", bufs=26))
        dpool = ctx.enter_context(tc.tile_pool(name="dpool", bufs=10))
        ppool = ctx.enter_context(tc.tile_pool(name="ppool", bufs=6, space="PSUM"))
        spool = ctx.enter_context(tc.tile_pool(name="spool", bufs=2, space="PSUM"))

        # ---- persistent SBUF ----
        zf_sb = wpool.tile([4, BC], F32)        # active-dim f32 state (output)
        zr_sb = wpool.tile([128, BC], F32R)     # duplicated matmul-operand mirror
        w1_sb = wpool.tile([128, HID], F32R)
        wf_sb = wpool.tile([128, 4 * 4 * HID], F32R)
        wb_sb = wpool.tile([128, 4 * 4 * HID], F32R)
        wga_sb = wpool.tile([128, 16 * 2 * NSTEP], F32R)
        w6_sb = wpool.tile([128, 4], F32)
        bia_sb = wpool.tile([128, 4 * NL], F32)
        smp_sb = wpool.tile([4, 4 * 2 * NSTEP], F32R)
        for name, t in (("zf", zf_sb), ("zr", zr_sb), ("w1", w1_sb),
                        ("wga", wga_sb), ("w6", w6_sb), ("bia", bia_sb),
                        ("smp", smp_sb)):
            nc.sync.dma_start(t[:], d[name][:])
        # per-layer weight DMAs so layer 2 can start before the whole 8MB lands;
        # backward weights in consumption order (l=5 first)
        for li in range(4):
            sl = slice(li * 4 * HID, (li + 1) * 4 * HID)
            nc.sync.dma_start(wf_sb[:, sl], d["wf"][:, sl])
        for li in range(3, -1, -1):
            sl = slice(li * 4 * HID, (li + 1) * 4 * HID)
            nc.sync.dma_start(wb_sb[:, sl], d["wb"][:, sl])

        def grad_active(btsl):
            """Forward+backward through the H-net for one batch tile.
            Returns the first-layer backward deltas d1[k] [128, BT] (k=0..3);
            contracting them with W1[:4,:].T gives the active-dim gradient."""
            # layer 1: K=64 contraction over the full latent. z and W1 are
            # duplicated across both partition halves, so pairs of matmuls run
            # concurrently on disjoint PE row groups.
            hprev = []
            tsaved = []  # tsaved[l-1][m] = h_l[m]^2
            t1 = []
            for m in range(4):
                base = 64 * (m % 2)
                ps = ppool.tile([128, BT], F32, tag="ps")
                nc.tensor.matmul(ps[:], w1_sb[base:base + 64, m * 128:(m + 1) * 128],
                                 zr_sb[base:base + 64, btsl], start=True, stop=True,
                                 tile_position=(base, 0))
                h = hpool.tile([128, BT], F32R, tag="h")
                nc.scalar.activation(h[:], ps[:], AF.Tanh, bias=bia_sb[:, m:m + 1])
                t = tpool.tile([128, BT], F32, tag="t")
                nc.vector.tensor_tensor(t[:], h[:], h[:], ALU.mult)
                hprev.append(h)
                t1.append(t)
            tsaved.append(t1)
            # layers 2..5
            for li in range(4):
                hcur = []
                tl = []
                for m in range(4):
                    ps = ppool.tile([128, BT], F32, tag="ps")
                    for k in range(4):
                        lhsT = wf_sb[:, li * 4 * HID + k * HID + m * 128:
                                     li * 4 * HID + k * HID + (m + 1) * 128]
                        nc.tensor.matmul(ps[:], lhsT, hprev[k][:],
                                         start=(k == 0), stop=(k == 3))
                    h = hpool.tile([128, BT], F32R, tag="h")
                    nc.scalar.activation(h[:], ps[:], AF.Tanh,
                                         bias=bia_sb[:, (li + 1) * 4 + m:(li + 1) * 4 + m + 1])
                    t = tpool.tile([128, BT], F32, tag="t")
                    nc.vector.tensor_tensor(t[:], h[:], h[:], ALU.mult)
                    hcur.append(h)
                    tl.append(t)
                tsaved.append(tl)
                hprev = hcur
            # backward. tsaved holds s = +h^2, so each (s-1)* flips the sign;
            # seed stores -d5, parity alternates, d1 comes out NEGATED and the
            # host-side wga is negated to compensate.
            dcur = []
            for m in range(4):
                dd = dpool.tile([128, BT], F32R, tag="d")
                nc.vector.tensor_scalar(dd[:], tsaved[4][m][:], 1.0,
                                        w6_sb[:, m:m + 1], ALU.subtract, ALU.mult)
                dcur.append(dd)
            # backward layers 5..2: d_{l-1}[k] = (s_{l-1}[k]-1) * (W_l @ d_l)[k]
            for li in range(3, -1, -1):
                dnew = []
                for k in range(4):
                    ps = ppool.tile([128, BT], F32, tag="ps")
                    for m in range(4):
                        lhsT = wb_sb[:, li * 4 * HID + m * HID + k * 128:
                                     li * 4 * HID + m * HID + (k + 1) * 128]
                        nc.tensor.matmul(ps[:], lhsT, dcur[m][:],
                                         start=(m == 0), stop=(m == 3))
                    dd = dpool.tile([128, BT], F32R, tag="d")
                    nc.vector.scalar_tensor_tensor(dd[:], tsaved[li][k][:], 1.0,
                                                   ps[:], ALU.subtract, ALU.mult)
                    dnew.append(dd)
                dcur = dnew
            return dcur

        def update(btsl, hs, d1):
            """Active-dim update. The whole update accumulates in one PSUM bank:
              znew = sum_k (cvec .* W1actTp)[k]^T @ d1[k]  +  A_hs^T @ z_active
            then ACT copies write the f32 state and both f32r mirror halves."""
            gps = spool.tile([4, BT], F32, tag="gps")
            for k in range(4):
                nc.tensor.matmul(gps[:], wga_sb[:, 16 * hs + 4 * k:16 * hs + 4 * k + 4],
                                 d1[k][:], start=(k == 0), stop=False)
            nc.tensor.matmul(gps[:], smp_sb[0:4, 4 * hs:4 * hs + 4],
                             zr_sb[0:4, btsl], start=False, stop=True)
            nc.scalar.activation(zf_sb[0:4, btsl], gps[:], AF.Copy)
            nc.scalar.activation(zr_sb[0:4, btsl], gps[:], AF.Copy)
            nc.scalar.activation(zr_sb[64:68, btsl], gps[:], AF.Copy)

        # interleave pairs of independent batch-tile chains so the PE can fill
        # each chain's update-dependency latency with the other chain's matmuls
        for p in range(0, n_bt, 2):
            slA = slice(p * BT, (p + 1) * BT)
            pair = p + 1 < n_bt
            slB = slice((p + 1) * BT, (p + 2) * BT) if pair else None
            for s in range(n_step):
                for half in range(2):
                    hs = 2 * s + half
                    update(slA, hs, grad_active(slA))
                    if pair:
                        update(slB, hs, grad_active(slB))
            nc.sync.dma_start(zoa[:, slA], zf_sb[:, slA])
            if pair:
                nc.sync.dma_start(zoa[:, slB], zf_sb[:, slB])
        # aux rows never change; ship them once from the mirror
        nc.sync.dma_start(zox[:], zr_sb[4:LAT, :])

    nc.compile()
    return nc


def _host_prep(z, W1, b1, W2, b2, W3, b3, W4, b4, W5, b5, W6, b6, S,
               dt_q, dt_p, alpha):
    """Build the per-core input maps (weight transforms are O(HID^2) only)."""
    a1c = 1.0 / (4.0 - 4.0 ** (1.0 / 3.0))
    a3c = 1.0 - 4.0 * a1c
    dts = [a * DT for a in (a1c, a1c, a3c, a1c, a1c)]
    dtq = float(np.asarray(dt_q).reshape(-1)[0])
    dtp = float(np.asarray(dt_p).reshape(-1)[0])
    al = float(np.asarray(alpha))
    S = np.asarray(S, np.float32)

    W1 = np.asarray(W1, np.float32)
    # swapped columns; negated to absorb the backward sign alternation
    wga_full = -W1[0:4, :].T[:, [2, 3, 0, 1]]  # [512, 4]

    smp = np.zeros((4, 4 * 2 * NSTEP), np.float32)
    wga = np.zeros((128, 16 * 2 * NSTEP), np.float32)
    eye = np.eye(4, dtype=np.float32)
    for s, dt in enumerate(dts):
        cg1 = dt * dtq            # scales dH/dz2 in the z1 update
        cg2 = -(dt / 2.0) * dtp   # scales dH/dz1 in the z2 update
        A = eye.copy()
        # z1_new cols: + alpha*dt*(z@S^T)[:, :2]  -> A[i,j] += al*dt*S[j,i], j<2
        A[:, 0:2] += al * dt * S[0:2, :].T
        # z2_new cols: + alpha*(dt/2)*(z@S)[:, 2:] -> A[i,j] += al*dt/2*S[i,j], j>=2
        A[:, 2:4] += al * (dt / 2.0) * S[:, 2:4]
        Ab = eye.copy()
        Ab[:, 2:4] = A[:, 2:4]
        smp[:, 4 * (2 * s):4 * (2 * s) + 4] = A
        smp[:, 4 * (2 * s + 1):4 * (2 * s + 1) + 4] = Ab
        cv0 = np.array([cg1, cg1, cg2, cg2], np.float32)
        cv1 = np.array([0.0, 0.0, cg2, cg2], np.float32)
        wga[:, 16 * (2 * s):16 * (2 * s) + 16] = _pack_k(wga_full * cv0[None, :])
        wga[:, 16 * (2 * s + 1):16 * (2 * s + 1) + 16] = _pack_k(wga_full * cv1[None, :])

    wf = np.concatenate([_pack_k(np.asarray(w, np.float32)) for w in (W2, W3, W4, W5)], axis=1)
    wb = np.concatenate([_pack_k(np.asarray(w, np.float32).T.copy()) for w in (W2, W3, W4, W5)], axis=1)
    w6p = np.asarray(W6, np.float32).reshape(4, 128).T.copy()  # [128,4], col k = W6[k*128:(k+1)*128]
    bia = np.zeros((128, 4 * NL), np.float32)
    for li, b in enumerate((b1, b2, b3, b4, b5)):
        bia[:, 4 * li:4 * li + 4] = np.asarray(b, np.float32).reshape(4, 128).T

    w1d = np.concatenate([W1, W1], axis=0)  # [128, 512], duplicated halves
    shared = {"w1": w1d, "wf": wf, "wb": wb, "wga": wga, "w6": w6p,
              "bia": bia, "smp": smp}
    z = np.asarray(z, np.float32)
    in_maps = []
    for c in range(N_CORES):
        zc = np.ascontiguousarray(z[c * BC:(c + 1) * BC, :].T)  # [64, 4096]
        m = dict(shared)
        m["zf"] = np.ascontiguousarray(zc[0:4, :])
        m["zr"] = np.concatenate([zc, zc], axis=0)  # [128, 4096]
        in_maps.append(m)
    return in_maps


_cached_nc = None


def kernel(z, W1, b1, W2, b2, W3, b3, W4, b4, W5, b5, W6, b6, S,
           dt_q, dt_p, alpha, _trace=False, _trace_kwargs=None):
    global _cached_nc
    in_maps = _host_prep(z, W1, b1, W2, b2, W3, b3, W4, b4, W5, b5, W6, b6, S,
                         dt_q, dt_p, alpha)
    if _cached_nc is None:
        _cached_nc = build_program()
    nc = _cached_nc
    res = run_bass_kernel_spmd(
        nc, in_maps, core_ids=list(range(N_CORES)), trace=_trace,
        **(_trace_kwargs or {}),
    )
    kernel.last_result = res
    out = np.empty((B, LAT), np.float32)
    for c in range(N_CORES):
        out[c * BC:(c + 1) * BC, 0:4] = res.results[c]["zoa"].T
        out[c * BC:(c + 1) * BC, 4:] = res.results[c]["zox"].T
    return out

